# revision 33
# baseline (speedup 1.0000x reference)
"""Multi-head attention block (B=8, S=1024, D=768, H=12) on 8 TRN2 NeuronCores.

Data-parallel: one batch element per core (attention is independent per batch).
Per-core pipeline (bf16 matmuls, fp32 PSUM accumulation):

  prologue: short HAM warm-up matmul burst from t=0; x as two contiguous
            ~0.75MB DMAs (sync + scalar queues); weights one ~1.2MB DMA each
            over sync/scalar/gpsimd queues; x -> xT via PE transpose (casts
            split across DVE and ACT); QT/KT chunk 0 only.
  QT = Wq^T xT (+bq), KT = Wk^T xT (+bk)      [D,S]  (head pairs per 128-chunk)
  V  = x Wv (+bv)                             [S,D]  stored as [128,12,65] with
                                                      a ones column per head
  per head pair (2c,2c+1):
      S^T pair = K Q^T row-packed on PE array halves -> [128,S] PSUM tiles
      exp(scale*S^T) -> PT pair (bf16, one ACT op per head per key-chunk)
      per head: O'^T = [V_h,1]^T PT (accum Sk) -> rows 0:64 = O^T, row 64 = rowsum
                OT_h = O'^T[0:64] * (1 / bcast(rowsum))
  out = O_cat @ Wp (+bp)                      [S,D]  (fp32, DMA halves on both
                                                      HWDGE queues)
"""

import numpy as np

B, S, DIM, H = 8, 1024, 768, 12
HD = DIM // H          # 64
SCALE = HD ** -0.5
N_CORES = 8
KC = DIM // 128        # 6 d-chunks
SC = S // 128          # 8 seq-chunks

_CACHE = {}


def _build():
    import concourse.mybir as mybir
    import concourse.tile as tile
    from concourse import bacc
    from concourse.masks import make_identity

    f32 = mybir.dt.float32
    bf16 = mybir.dt.bfloat16
    EXP = mybir.ActivationFunctionType.Exp

    nc = bacc.Bacc()

    x_ext = nc.declare_dram_parameter("x", [S, DIM], bf16, isOutput=False)
    Wq_ext = nc.declare_dram_parameter("Wq", [DIM, DIM], bf16, isOutput=False)
    bq_ext = nc.declare_dram_parameter("bq", [DIM], f32, isOutput=False)
    Wk_ext = nc.declare_dram_parameter("Wk", [DIM, DIM], bf16, isOutput=False)
    bk_ext = nc.declare_dram_parameter("bk", [DIM], f32, isOutput=False)
    Wv_ext = nc.declare_dram_parameter("Wv", [DIM, DIM], bf16, isOutput=False)
    bv_ext = nc.declare_dram_parameter("bv", [DIM], bf16, isOutput=False)
    Wp_ext = nc.declare_dram_parameter("Wp", [DIM, DIM], bf16, isOutput=False)
    bp_ext = nc.declare_dram_parameter("bp", [DIM], bf16, isOutput=False)
    out_ext = nc.declare_dram_parameter("out", [S, DIM], bf16, isOutput=True)

    HALVES = ((0, 512), (512, 1024))
    VHALVES = ((0, 512), (512, DIM))

    with tile.TileContext(nc) as tc:
        with tc.tile_pool(name="persist", bufs=1) as sb, \
             tc.tile_pool(name="ps", bufs=1, space="PSUM") as ps:

            def p2(name, shape=(128, S), dtype=f32):
                return ps.tile(list(shape), dtype, tag="p2", bufs=2, name=name)

            # ---- constants (gpsimd identity first: its queue also carries
            # the SWDGE weight DMA later) ----
            ident = sb.tile([128, 128], bf16)
            make_identity(nc, ident)
            ones2d = sb.tile([128, 128], bf16)
            nc.vector.memset(ones2d, 1.0)
            rs_z = sb.tile([128, S], bf16)
            nc.vector.memset(rs_z, 0.0)
            V12 = [sb.tile([128, H, HD + 1], bf16, name=f"V12_{s8}") for s8 in range(SC)]
            for s8 in range(SC):
                nc.vector.memset(V12[s8][:, :, HD:HD + 1], 1.0)

            # HAM warm-up: a short dummy-matmul burst from t=0 (while the
            # input DMAs run) flips the PE clock-gate to 8/8 just before the
            # transposes arrive; the dense transpose+QK stream keeps it warm.
            for _ in range(6):
                wu_ps = p2("wu_ps")
                for n0, n1 in HALVES:
                    nc.tensor.matmul(wu_ps[:, n0:n1], ones2d,
                                     rs_z[:, n0:n1], start=True, stop=True)

            # ---- input DMAs over three queues ----
            x_sb = sb.tile([128, SC, DIM], bf16, name="x_sb")
            nc.sync.dma_start(
                out=x_sb[:, 0:4, :],
                in_=x_ext[0:512, :].rearrange("(a p) n -> p a n", p=128))
            nc.scalar.dma_start(
                out=x_sb[:, 4:8, :],
                in_=x_ext[512:1024, :].rearrange("(a p) n -> p a n", p=128))

            bq_sb = sb.tile([128, KC], f32)
            nc.gpsimd.dma_start(out=bq_sb, in_=bq_ext[:].rearrange("(c p) -> p c", p=128))
            bk_sb = sb.tile([128, KC], f32)
            nc.gpsimd.dma_start(out=bk_sb, in_=bk_ext[:].rearrange("(c p) -> p c", p=128))
            bv_row = sb.tile([1, DIM], bf16)
            nc.gpsimd.dma_start(out=bv_row, in_=bv_ext[:].rearrange("(a d) -> a d", a=1))
            bp_row = sb.tile([1, DIM], bf16)
            nc.gpsimd.dma_start(out=bp_row, in_=bp_ext[:].rearrange("(a d) -> a d", a=1))

            def w_load(W_ext, eng, name):
                w = sb.tile([128, KC, DIM], bf16, name=name)
                eng.dma_start(
                    out=w, in_=W_ext[:].rearrange("(c p) n -> p c n", p=128))
                return [w[:, c, :] for c in range(KC)]

            Wq_sb = w_load(Wq_ext, nc.scalar, "Wq")
            Wk_sb = w_load(Wk_ext, nc.sync, "Wk")
            Wv_sb = w_load(Wv_ext, nc.scalar, "Wv")
            Wp_sb = w_load(Wp_ext, nc.gpsimd, "Wp")

            # x -> xT via PE transpose; PSUM->SBUF casts split DVE/ACT
            xT = [sb.tile([128, S], bf16, name=f"xT{c}") for c in range(KC)]
            for c in range(KC):
                xt_ps = p2("xt_ps", (128, S), bf16)
                for s8 in range(SC):
                    nc.tensor.transpose(
                        xt_ps[:, s8 * 128:(s8 + 1) * 128],
                        x_sb[:, s8, c * 128:(c + 1) * 128], ident)
                nc.vector.tensor_copy(xT[c][:, 0:512], xt_ps[:, 0:512])
                nc.scalar.copy(xT[c][:, 512:1024], xt_ps[:, 512:1024])

            # persistent activations
            QT = [sb.tile([128, S], bf16, name=f"QT{c}") for c in range(KC)]
            KT = [sb.tile([128, S], bf16, name=f"KT{c}") for c in range(KC)]

            # QT / KT chunk m: out[d_out, seq] = W^T @ xT, bias per partition
            def qk_chunk(W_sb, bias_sb, dst, m):
                q_ps = p2("q_ps")
                for k in range(KC):
                    for n0, n1 in HALVES:
                        nc.tensor.matmul(
                            q_ps[:, n0:n1],
                            W_sb[k][:, m * 128:(m + 1) * 128],
                            xT[k][:, n0:n1],
                            start=(k == 0), stop=(k == KC - 1))
                nc.vector.tensor_scalar_add(dst[m], q_ps, bias_sb[:, m:m + 1])

            qk_chunk(Wq_sb, bq_sb, QT, 0)
            qk_chunk(Wk_sb, bk_sb, KT, 0)

            # broadcast bv/bp across partitions (K=128 matmul on the
            # zero-padded carrier; row 0 = bias row); after qk0 so nothing
            # in the PE FIFO waits on the slow gpsimd bias DMAs early on.
            bv_bc = sb.tile([128, DIM], f32)
            bp_bc = sb.tile([128, DIM], f32)
            for row, bc in ((bv_row, bv_bc), (bp_row, bp_bc)):
                nc.vector.tensor_copy(rs_z[0:1, 0:DIM], row[0:1, :])
                bc_ps = p2("bias_ps")
                for n0, n1 in VHALVES:
                    nc.tensor.matmul(bc_ps[:, n0:n1], ones2d,
                                     rs_z[:, n0:n1], start=True, stop=True)
                # DVE, not ACT: the strict-FIFO ACT queue must stay clear for
                # the first softmax exps
                nc.vector.tensor_copy(bc, bc_ps[:, 0:DIM])

            # V natural layout: out[seq, d] = x Wv (+bv); trickled into
            # pair 0's kc loop to fill the exp-paced PE slack.
            def emit_v_chunk(s8):
                v_ps = p2("v_ps")
                for k in range(KC):
                    for n0, n1 in VHALVES:
                        nc.tensor.matmul(
                            v_ps[:, n0:n1],
                            xT[k][:, s8 * 128:(s8 + 1) * 128],
                            Wv_sb[k][:, n0:n1],
                            start=(k == 0), stop=(k == KC - 1))
                nc.vector.tensor_add(
                    V12[s8][:, :, 0:HD],
                    v_ps[:, 0:DIM].rearrange("p (h d) -> p h d", h=H),
                    bv_bc[:].rearrange("p (h d) -> p h d", h=H))

            # ---- attention + output projection ----
            with tc.tile_pool(name="pb", bufs=1) as pb:
                OT = [pb.tile([128, S], bf16, name=f"OT{c}") for c in range(KC)]

                def norm_head(ov, c, half):
                    # rowsum (into row 0 of the zero-padded carrier) ->
                    # broadcast via full-util K=128 matmul -> 1/x -> normalize
                    nc.vector.tensor_copy(rs_z[0:1, :], ov[HD:HD + 1, :])
                    bc_ps = ps.tile([128, S], f32, tag="p2", bufs=2, name="bc_ps")
                    for n0, n1 in HALVES:
                        nc.tensor.matmul(bc_ps[:, n0:n1], ones2d,
                                         rs_z[:, n0:n1], start=True, stop=True)
                    rbc = pb.tile([HD, S], f32, tag="rbc", bufs=2, name="rbc")
                    nc.vector.reciprocal_approx_fast(rbc, bc_ps[0:HD, :])
                    base = half * HD
                    nc.vector.tensor_mul(OT[c][base:base + HD, :], ov[0:HD, :], rbc)

                def s_exp_part(c, pt, kcs, with_v=False, qk=None):
                    # S^T for head pair (2c, 2c+1): row-packed matmuls on array
                    # halves run concurrently; exp'd scores buffer in SBUF.
                    for kc in kcs:
                        st_e = ps.tile([128, S], f32, tag="st2", bufs=2, name="st_e")
                        st_o = ps.tile([128, S], f32, tag="st2", bufs=2, name="st_o")
                        for n0, n1 in HALVES:
                            nc.tensor.matmul(
                                st_e[:, n0:n1],
                                KT[c][0:HD, kc * 128:(kc + 1) * 128],
                                QT[c][0:HD, n0:n1],
                                start=True, stop=True)
                            nc.tensor.matmul(
                                st_o[:, n0:n1],
                                KT[c][HD:128, kc * 128:(kc + 1) * 128],
                                QT[c][HD:128, n0:n1],
                                start=True, stop=True)
                        p_e = pb.tile([128, S], bf16, tag=f"pt{kc}e", bufs=2, name=f"pt{kc}e")
                        nc.scalar.activation(p_e, st_e, EXP, scale=SCALE)
                        p_o = pb.tile([128, S], bf16, tag=f"pt{kc}o", bufs=2, name=f"pt{kc}o")
                        nc.scalar.activation(p_o, st_o, EXP, scale=SCALE)
                        pt[0].append(p_e)
                        pt[1].append(p_o)
                        if with_v:
                            emit_v_chunk(kc)
                        if qk is not None and kc in qk[1]:
                            # QK chunk for the *next* pair, emitted inside
                            # this pair's exp-paced stretch: the next pair's
                            # scores then exist before ACT runs dry.
                            m = qk[0]
                            if qk[1][kc] == 'q':
                                qk_chunk(Wq_sb, bq_sb, QT, m)
                            else:
                                qk_chunk(Wk_sb, bk_sb, KT, m)
                    return pt

                def pv_norm(c, half, ptl):
                    ov = ps.tile([HD + 1, S], f32, tag="p2", bufs=2, name="ov")
                    for kc in range(SC):
                        for n0, n1 in HALVES:
                            nc.tensor.matmul(
                                ov[:, n0:n1],
                                V12[kc][:, 2 * c + half, :],
                                ptl[kc][:, n0:n1],
                                start=(kc == 0), stop=(kc == SC - 1))
                    norm_head(ov, c, half)

                # Half-skewed pipeline (the ordering this hardware likes),
                # with a 2/6 split of each score stream across the iteration
                # boundary: key-chunks 0-1 of pair c+2 are emitted right
                # after pv(c,1), so ACT stays fed through the boundary while
                # the PE runs the next iteration's PV front.  QK chunks ride
                # inside the streams one pair ahead of use.
                pts_cur = ([], [])
                s_exp_part(0, pts_cur, range(SC), with_v=True,
                           qk=(1, {1: 'q', 4: 'k'}))
                pts_nxt = ([], [])
                s_exp_part(1, pts_nxt, range(0, 2))
                for c in range(KC):
                    pv_norm(c, 0, pts_cur[0])
                    if c + 1 < KC:
                        s_exp_part(c + 1, pts_nxt, range(2, SC),
                                   qk=((c + 2, {2: 'q', 5: 'k'})
                                       if c + 2 < KC else None))
                    pv_norm(c, 1, pts_cur[1])
                    pts_fut = ([], [])
                    if c + 2 < KC:
                        s_exp_part(c + 2, pts_fut, range(0, 2))
                    pts_cur, pts_nxt = pts_nxt, pts_fut

                # out = O_cat @ Wp + bp; writeback halves on both HWDGE queues
                for s8 in range(SC):
                    f_ps = p2("f_ps")
                    for k in range(KC):
                        for n0, n1 in VHALVES:
                            nc.tensor.matmul(
                                f_ps[:, n0:n1],
                                OT[k][:, s8 * 128:(s8 + 1) * 128],
                                Wp_sb[k][:, n0:n1],
                                start=(k == 0), stop=(k == KC - 1))
                    # bf16 writeback (host upcasts): halves the output-DMA
                    # queue time that otherwise drains past the last matmul
                    fin = pb.tile([128, DIM], bf16, tag="fin", bufs=2, name="fin")
                    nc.vector.tensor_add(fin, f_ps[:, 0:DIM], bp_bc)
                    e0, e1 = (nc.sync, nc.scalar) if s8 % 2 == 0 else (nc.scalar, nc.sync)
                    e0.dma_start(out=out_ext[s8 * 128:(s8 + 1) * 128, 0:512],
                                 in_=fin[:, 0:512])
                    e1.dma_start(out=out_ext[s8 * 128:(s8 + 1) * 128, 512:DIM],
                                 in_=fin[:, 512:DIM])

    nc.compile()
    return nc


def get_nc():
    if "nc" not in _CACHE:
        _CACHE["nc"] = _build()
    return _CACHE["nc"]


def kernel(x, Wq, bq, Wk, bk, Wv, bv, Wp, bp):
    import ml_dtypes
    from concourse.bass_utils import run_bass_kernel_spmd

    nc = get_nc()
    bfl = ml_dtypes.bfloat16
    x = np.ascontiguousarray(np.asarray(x, np.float32).astype(bfl))
    shared = {
        "Wq": np.ascontiguousarray(np.asarray(Wq, np.float32).astype(bfl)),
        "bq": np.ascontiguousarray(np.asarray(bq, np.float32)),
        "Wk": np.ascontiguousarray(np.asarray(Wk, np.float32).astype(bfl)),
        "bk": np.ascontiguousarray(np.asarray(bk, np.float32)),
        "Wv": np.ascontiguousarray(np.asarray(Wv, np.float32).astype(bfl)),
        "bv": np.ascontiguousarray(np.asarray(bv, np.float32).astype(bfl)),
        "Wp": np.ascontiguousarray(np.asarray(Wp, np.float32).astype(bfl)),
        "bp": np.ascontiguousarray(np.asarray(bp, np.float32).astype(bfl)),
    }
    in_maps = [{"x": x[b], **shared} for b in range(N_CORES)]
    res = run_bass_kernel_spmd(nc, in_maps, core_ids=list(range(N_CORES)))
    return np.stack([res.results[i]["out"] for i in range(N_CORES)],
                    axis=0).astype(np.float32)


# revision 37
# speedup vs baseline: 1.0108x; 1.0108x over previous
"""Multi-head attention block (B=8, S=1024, D=768, H=12) on 8 TRN2 NeuronCores.

Data-parallel: one batch element per core (attention is independent per batch).
Per-core pipeline (bf16 matmuls, fp32 PSUM accumulation):

  prologue: short HAM warm-up matmul burst from t=0; x as two contiguous
            ~0.75MB DMAs (sync + scalar queues); weights one ~1.2MB DMA each
            over sync/scalar/gpsimd queues; x -> xT via PE transpose (casts
            split across DVE and ACT); QT/KT chunk 0 only.
  QT = Wq^T xT (+bq), KT = Wk^T xT (+bk)      [D,S]  (head pairs per 128-chunk)
  V  = x Wv (+bv)                             [S,D]  stored as [128,12,65] with
                                                      a ones column per head
  per head pair (2c,2c+1):
      S^T pair = K Q^T row-packed on PE array halves -> [128,S] PSUM tiles
      exp(scale*S^T) -> PT pair (bf16, one ACT op per head per key-chunk)
      per head: O'^T = [V_h,1]^T PT (accum Sk) -> rows 0:64 = O^T, row 64 = rowsum
                OT_h = O'^T[0:64] * (1 / bcast(rowsum))
  out = O_cat @ Wp (+bp)                      [S,D]  (fp32, DMA halves on both
                                                      HWDGE queues)
"""

import numpy as np

B, S, DIM, H = 8, 1024, 768, 12
HD = DIM // H          # 64
SCALE = HD ** -0.5
N_CORES = 8
KC = DIM // 128        # 6 d-chunks
SC = S // 128          # 8 seq-chunks

_CACHE = {}


def _build():
    import concourse.mybir as mybir
    import concourse.tile as tile
    from concourse import bacc
    from concourse.masks import make_identity

    f32 = mybir.dt.float32
    bf16 = mybir.dt.bfloat16
    EXP = mybir.ActivationFunctionType.Exp

    nc = bacc.Bacc()

    x_ext = nc.declare_dram_parameter("x", [S, DIM], bf16, isOutput=False)
    Wq_ext = nc.declare_dram_parameter("Wq", [DIM, DIM], bf16, isOutput=False)
    bq_ext = nc.declare_dram_parameter("bq", [DIM], f32, isOutput=False)
    Wk_ext = nc.declare_dram_parameter("Wk", [DIM, DIM], bf16, isOutput=False)
    bk_ext = nc.declare_dram_parameter("bk", [DIM], f32, isOutput=False)
    Wv_ext = nc.declare_dram_parameter("Wv", [DIM, DIM], bf16, isOutput=False)
    bv_ext = nc.declare_dram_parameter("bv", [DIM], bf16, isOutput=False)
    Wp_ext = nc.declare_dram_parameter("Wp", [DIM, DIM], bf16, isOutput=False)
    bp_ext = nc.declare_dram_parameter("bp", [DIM], bf16, isOutput=False)
    out_ext = nc.declare_dram_parameter("out", [S, DIM], bf16, isOutput=True)

    HALVES = ((0, 512), (512, 1024))
    VHALVES = ((0, 512), (512, DIM))

    with tile.TileContext(nc) as tc:
        with tc.tile_pool(name="persist", bufs=1) as sb, \
             tc.tile_pool(name="ps", bufs=1, space="PSUM") as ps:

            def p2(name, shape=(128, S), dtype=f32):
                return ps.tile(list(shape), dtype, tag="p2", bufs=2, name=name)

            # ---- constants (gpsimd identity first: its queue also carries
            # the SWDGE weight DMA later) ----
            ident = sb.tile([128, 128], bf16)
            make_identity(nc, ident)
            ones2d = sb.tile([128, 128], bf16)
            nc.vector.memset(ones2d, 1.0)
            rs_z = sb.tile([128, S], bf16)
            nc.vector.memset(rs_z, 0.0)
            V12 = [sb.tile([128, H, HD + 1], bf16, name=f"V12_{s8}") for s8 in range(SC)]
            for s8 in range(SC):
                nc.vector.memset(V12[s8][:, :, HD:HD + 1], 1.0)

            # HAM warm-up: a short dummy-matmul burst from t=0 (while the
            # input DMAs run) flips the PE clock-gate to 8/8 just before the
            # transposes arrive; the dense transpose+QK stream keeps it warm.
            for _ in range(6):
                wu_ps = p2("wu_ps")
                for n0, n1 in HALVES:
                    nc.tensor.matmul(wu_ps[:, n0:n1], ones2d,
                                     rs_z[:, n0:n1], start=True, stop=True)

            # ---- input DMAs over three queues ----
            x_sb = sb.tile([128, SC, DIM], bf16, name="x_sb")
            nc.sync.dma_start(
                out=x_sb[:, 0:4, :],
                in_=x_ext[0:512, :].rearrange("(a p) n -> p a n", p=128))
            nc.scalar.dma_start(
                out=x_sb[:, 4:8, :],
                in_=x_ext[512:1024, :].rearrange("(a p) n -> p a n", p=128))

            bq_sb = sb.tile([128, KC], f32)
            nc.gpsimd.dma_start(out=bq_sb, in_=bq_ext[:].rearrange("(c p) -> p c", p=128))
            bk_sb = sb.tile([128, KC], f32)
            nc.gpsimd.dma_start(out=bk_sb, in_=bk_ext[:].rearrange("(c p) -> p c", p=128))
            bv_row = sb.tile([1, DIM], bf16)
            nc.gpsimd.dma_start(out=bv_row, in_=bv_ext[:].rearrange("(a d) -> a d", a=1))
            bp_row = sb.tile([1, DIM], bf16)
            nc.gpsimd.dma_start(out=bp_row, in_=bp_ext[:].rearrange("(a d) -> a d", a=1))

            def w_load(W_ext, eng, name):
                w = sb.tile([128, KC, DIM], bf16, name=name)
                eng.dma_start(
                    out=w, in_=W_ext[:].rearrange("(c p) n -> p c n", p=128))
                return [w[:, c, :] for c in range(KC)]

            Wq_sb = w_load(Wq_ext, nc.scalar, "Wq")
            Wk_sb = w_load(Wk_ext, nc.sync, "Wk")
            Wv_sb = w_load(Wv_ext, nc.scalar, "Wv")
            Wp_sb = w_load(Wp_ext, nc.gpsimd, "Wp")

            # x -> xT via PE transpose; PSUM->SBUF casts split DVE/ACT
            xT = [sb.tile([128, S], bf16, name=f"xT{c}") for c in range(KC)]
            for c in range(KC):
                xt_ps = p2("xt_ps", (128, S), bf16)
                for s8 in range(SC):
                    nc.tensor.transpose(
                        xt_ps[:, s8 * 128:(s8 + 1) * 128],
                        x_sb[:, s8, c * 128:(c + 1) * 128], ident)
                nc.vector.tensor_copy(xT[c][:, 0:512], xt_ps[:, 0:512])
                nc.scalar.copy(xT[c][:, 512:1024], xt_ps[:, 512:1024])

            # persistent activations
            QT = [sb.tile([128, S], bf16, name=f"QT{c}") for c in range(KC)]
            KT = [sb.tile([128, S], bf16, name=f"KT{c}") for c in range(KC)]

            # QT / KT chunk m: out[d_out, seq] = W^T @ xT, bias per partition
            def qk_chunk(W_sb, bias_sb, dst, m):
                q_ps = p2("q_ps")
                for k in range(KC):
                    for n0, n1 in HALVES:
                        nc.tensor.matmul(
                            q_ps[:, n0:n1],
                            W_sb[k][:, m * 128:(m + 1) * 128],
                            xT[k][:, n0:n1],
                            start=(k == 0), stop=(k == KC - 1))
                nc.vector.tensor_scalar_add(dst[m], q_ps, bias_sb[:, m:m + 1])

            # half-width QK piece: 6 matmuls, sized to fit the ~1.3us of PE
            # slack one exp-paced key-chunk slot offers
            def qk_half(W_sb, bias_sb, dst, m, half):
                n0, n1 = HALVES[half]
                q_ps = p2("qk_h", (128, 512))
                for k in range(KC):
                    nc.tensor.matmul(
                        q_ps, W_sb[k][:, m * 128:(m + 1) * 128],
                        xT[k][:, n0:n1],
                        start=(k == 0), stop=(k == KC - 1))
                nc.vector.tensor_scalar_add(dst[m][:, n0:n1], q_ps,
                                            bias_sb[:, m:m + 1])

            qk_chunk(Wq_sb, bq_sb, QT, 0)
            qk_chunk(Wk_sb, bk_sb, KT, 0)

            # broadcast bv/bp across partitions (K=128 matmul on the
            # zero-padded carrier; row 0 = bias row); after qk0 so nothing
            # in the PE FIFO waits on the slow gpsimd bias DMAs early on.
            bv_bc = sb.tile([128, DIM], f32)
            bp_bc = sb.tile([128, DIM], f32)
            for row, bc in ((bv_row, bv_bc), (bp_row, bp_bc)):
                nc.vector.tensor_copy(rs_z[0:1, 0:DIM], row[0:1, :])
                bc_ps = p2("bias_ps")
                for n0, n1 in VHALVES:
                    nc.tensor.matmul(bc_ps[:, n0:n1], ones2d,
                                     rs_z[:, n0:n1], start=True, stop=True)
                # DVE, not ACT: the strict-FIFO ACT queue must stay clear for
                # the first softmax exps
                nc.vector.tensor_copy(bc, bc_ps[:, 0:DIM])

            # V natural layout: out[seq, d] = x Wv (+bv); trickled into
            # pair 0's kc loop to fill the exp-paced PE slack.
            def emit_v_chunk(s8):
                v_ps = p2("v_ps")
                for k in range(KC):
                    for n0, n1 in VHALVES:
                        nc.tensor.matmul(
                            v_ps[:, n0:n1],
                            xT[k][:, s8 * 128:(s8 + 1) * 128],
                            Wv_sb[k][:, n0:n1],
                            start=(k == 0), stop=(k == KC - 1))
                nc.vector.tensor_add(
                    V12[s8][:, :, 0:HD],
                    v_ps[:, 0:DIM].rearrange("p (h d) -> p h d", h=H),
                    bv_bc[:].rearrange("p (h d) -> p h d", h=H))

            # ---- attention + output projection ----
            with tc.tile_pool(name="pb", bufs=1) as pb:
                OT = [pb.tile([128, S], bf16, name=f"OT{c}") for c in range(KC)]

                def norm_head(ov, c, half):
                    # rowsum (into row 0 of the zero-padded carrier) ->
                    # broadcast via full-util K=128 matmul -> 1/x -> normalize
                    nc.vector.tensor_copy(rs_z[0:1, :], ov[HD:HD + 1, :])
                    bc_ps = ps.tile([128, S], f32, tag="p2", bufs=2, name="bc_ps")
                    for n0, n1 in HALVES:
                        nc.tensor.matmul(bc_ps[:, n0:n1], ones2d,
                                         rs_z[:, n0:n1], start=True, stop=True)
                    rbc = pb.tile([HD, S], f32, tag="rbc", bufs=2, name="rbc")
                    nc.vector.reciprocal_approx_fast(rbc, bc_ps[0:HD, :])
                    base = half * HD
                    nc.vector.tensor_mul(OT[c][base:base + HD, :], ov[0:HD, :], rbc)

                def s_exp_part(c, pt, kcs, with_v=False, qk=None):
                    # S^T for head pair (2c, 2c+1): row-packed matmuls on array
                    # halves run concurrently; exp'd scores buffer in SBUF.
                    for kc in kcs:
                        st_e = ps.tile([128, S], f32, tag="st2", bufs=2, name="st_e")
                        st_o = ps.tile([128, S], f32, tag="st2", bufs=2, name="st_o")
                        for n0, n1 in HALVES:
                            nc.tensor.matmul(
                                st_e[:, n0:n1],
                                KT[c][0:HD, kc * 128:(kc + 1) * 128],
                                QT[c][0:HD, n0:n1],
                                start=True, stop=True)
                            nc.tensor.matmul(
                                st_o[:, n0:n1],
                                KT[c][HD:128, kc * 128:(kc + 1) * 128],
                                QT[c][HD:128, n0:n1],
                                start=True, stop=True)
                        p_e = pb.tile([128, S], bf16, tag=f"pt{kc}e", bufs=2, name=f"pt{kc}e")
                        nc.scalar.activation(p_e, st_e, EXP, scale=SCALE)
                        p_o = pb.tile([128, S], bf16, tag=f"pt{kc}o", bufs=2, name=f"pt{kc}o")
                        nc.scalar.activation(p_o, st_o, EXP, scale=SCALE)
                        pt[0].append(p_e)
                        pt[1].append(p_o)
                        if with_v:
                            emit_v_chunk(kc)
                        if qk is not None and kc in qk[1]:
                            # QK piece for the *next* pair, emitted inside
                            # this pair's exp-paced stretch: the next pair's
                            # scores then exist before ACT runs dry.
                            m, kind = qk[0], qk[1][kc]
                            if kind[0] == 'q':
                                qk_half(Wq_sb, bq_sb, QT, m, int(kind[1]))
                            else:
                                qk_half(Wk_sb, bk_sb, KT, m, int(kind[1]))
                    return pt

                def pv_norm(c, half, ptl):
                    ov = ps.tile([HD + 1, S], f32, tag="p2", bufs=2, name="ov")
                    for kc in range(SC):
                        for n0, n1 in HALVES:
                            nc.tensor.matmul(
                                ov[:, n0:n1],
                                V12[kc][:, 2 * c + half, :],
                                ptl[kc][:, n0:n1],
                                start=(kc == 0), stop=(kc == SC - 1))
                    norm_head(ov, c, half)

                # Half-skewed pipeline (the ordering this hardware likes),
                # with a 2/6 split of each score stream across the iteration
                # boundary: key-chunks 0-1 of pair c+2 are emitted right
                # after pv(c,1), so ACT stays fed through the boundary while
                # the PE runs the next iteration's PV front.  QK chunks ride
                # inside the streams one pair ahead of use.
                pts_cur = ([], [])
                s_exp_part(0, pts_cur, range(SC), with_v=True,
                           qk=(1, {1: 'q0', 2: 'q1', 4: 'k0', 5: 'k1'}))
                pts_nxt = ([], [])
                s_exp_part(1, pts_nxt, range(0, 2))
                for c in range(KC):
                    pv_norm(c, 0, pts_cur[0])
                    if c + 1 < KC:
                        s_exp_part(c + 1, pts_nxt, range(2, SC),
                                   qk=((c + 2, {2: 'q0', 3: 'q1', 5: 'k0', 6: 'k1'})
                                       if c + 2 < KC else None))
                    pv_norm(c, 1, pts_cur[1])
                    pts_fut = ([], [])
                    if c + 2 < KC:
                        s_exp_part(c + 2, pts_fut, range(0, 2))
                    pts_cur, pts_nxt = pts_nxt, pts_fut

                # out = O_cat @ Wp + bp; writeback halves on both HWDGE queues
                for s8 in range(SC):
                    f_ps = p2("f_ps")
                    for k in range(KC):
                        for n0, n1 in VHALVES:
                            nc.tensor.matmul(
                                f_ps[:, n0:n1],
                                OT[k][:, s8 * 128:(s8 + 1) * 128],
                                Wp_sb[k][:, n0:n1],
                                start=(k == 0), stop=(k == KC - 1))
                    # bf16 writeback (host upcasts): halves the output-DMA
                    # queue time that otherwise drains past the last matmul
                    fin = pb.tile([128, DIM], bf16, tag="fin", bufs=2, name="fin")
                    nc.vector.tensor_add(fin, f_ps[:, 0:DIM], bp_bc)
                    e0, e1 = (nc.sync, nc.scalar) if s8 % 2 == 0 else (nc.scalar, nc.sync)
                    e0.dma_start(out=out_ext[s8 * 128:(s8 + 1) * 128, 0:512],
                                 in_=fin[:, 0:512])
                    e1.dma_start(out=out_ext[s8 * 128:(s8 + 1) * 128, 512:DIM],
                                 in_=fin[:, 512:DIM])

    nc.compile()
    return nc


def get_nc():
    if "nc" not in _CACHE:
        _CACHE["nc"] = _build()
    return _CACHE["nc"]


def kernel(x, Wq, bq, Wk, bk, Wv, bv, Wp, bp):
    import ml_dtypes
    from concourse.bass_utils import run_bass_kernel_spmd

    nc = get_nc()
    bfl = ml_dtypes.bfloat16
    x = np.ascontiguousarray(np.asarray(x, np.float32).astype(bfl))
    shared = {
        "Wq": np.ascontiguousarray(np.asarray(Wq, np.float32).astype(bfl)),
        "bq": np.ascontiguousarray(np.asarray(bq, np.float32)),
        "Wk": np.ascontiguousarray(np.asarray(Wk, np.float32).astype(bfl)),
        "bk": np.ascontiguousarray(np.asarray(bk, np.float32)),
        "Wv": np.ascontiguousarray(np.asarray(Wv, np.float32).astype(bfl)),
        "bv": np.ascontiguousarray(np.asarray(bv, np.float32).astype(bfl)),
        "Wp": np.ascontiguousarray(np.asarray(Wp, np.float32).astype(bfl)),
        "bp": np.ascontiguousarray(np.asarray(bp, np.float32).astype(bfl)),
    }
    in_maps = [{"x": x[b], **shared} for b in range(N_CORES)]
    res = run_bass_kernel_spmd(nc, in_maps, core_ids=list(range(N_CORES)))
    return np.stack([res.results[i]["out"] for i in range(N_CORES)],
                    axis=0).astype(np.float32)


# revision 39
# speedup vs baseline: 1.0303x; 1.0193x over previous
"""Multi-head attention block (B=8, S=1024, D=768, H=12) on 8 TRN2 NeuronCores.

Data-parallel: one batch element per core (attention is independent per batch).
Per-core pipeline (bf16 matmuls, fp32 PSUM accumulation):

  prologue: short HAM warm-up matmul burst from t=0; x as two contiguous
            ~0.75MB DMAs (sync + scalar queues); weights one ~1.2MB DMA each
            over sync/scalar/gpsimd queues; x -> xT via PE transpose (casts
            split across DVE and ACT); QT/KT chunk 0 only.
  QT = Wq^T xT (+bq), KT = Wk^T xT (+bk)      [D,S]  (head pairs per 128-chunk)
  V  = x Wv (+bv)                             [S,D]  stored as [128,12,65] with
                                                      a ones column per head
  per head pair (2c,2c+1):
      S^T pair = K Q^T row-packed on PE array halves -> [128,S] PSUM tiles
      exp(scale*S^T) -> PT pair (bf16, one ACT op per head per key-chunk)
      per head: O'^T = [V_h,1]^T PT (accum Sk) -> rows 0:64 = O^T, row 64 = rowsum
                OT_h = O'^T[0:64] * (1 / bcast(rowsum))
  out = O_cat @ Wp (+bp)                      [S,D]  (fp32, DMA halves on both
                                                      HWDGE queues)
"""

import numpy as np

B, S, DIM, H = 8, 1024, 768, 12
HD = DIM // H          # 64
SCALE = HD ** -0.5
N_CORES = 8
KC = DIM // 128        # 6 d-chunks
SC = S // 128          # 8 seq-chunks

_CACHE = {}


def _build():
    import concourse.mybir as mybir
    import concourse.tile as tile
    from concourse import bacc
    from concourse.masks import make_identity

    f32 = mybir.dt.float32
    bf16 = mybir.dt.bfloat16
    EXP = mybir.ActivationFunctionType.Exp

    nc = bacc.Bacc()

    x_ext = nc.declare_dram_parameter("x", [S, DIM], bf16, isOutput=False)
    Wq_ext = nc.declare_dram_parameter("Wq", [DIM, DIM], bf16, isOutput=False)
    bq_ext = nc.declare_dram_parameter("bq", [DIM], f32, isOutput=False)
    Wk_ext = nc.declare_dram_parameter("Wk", [DIM, DIM], bf16, isOutput=False)
    bk_ext = nc.declare_dram_parameter("bk", [DIM], f32, isOutput=False)
    Wv_ext = nc.declare_dram_parameter("Wv", [DIM, DIM], bf16, isOutput=False)
    bv_ext = nc.declare_dram_parameter("bv", [DIM], bf16, isOutput=False)
    Wp_ext = nc.declare_dram_parameter("Wp", [DIM, DIM], bf16, isOutput=False)
    bp_ext = nc.declare_dram_parameter("bp", [DIM], bf16, isOutput=False)
    out_ext = nc.declare_dram_parameter("out", [S, DIM], bf16, isOutput=True)

    HALVES = ((0, 512), (512, 1024))
    VHALVES = ((0, 512), (512, DIM))

    with tile.TileContext(nc) as tc:
        with tc.tile_pool(name="persist", bufs=1) as sb, \
             tc.tile_pool(name="ps", bufs=1, space="PSUM") as ps:

            def p2(name, shape=(128, S), dtype=f32):
                return ps.tile(list(shape), dtype, tag="p2", bufs=2, name=name)

            # ---- constants (gpsimd identity first: its queue also carries
            # the SWDGE weight DMA later) ----
            ident = sb.tile([128, 128], bf16)
            make_identity(nc, ident)
            ones2d = sb.tile([128, 128], bf16)
            nc.vector.memset(ones2d, 1.0)
            rs_z = sb.tile([128, S], bf16)
            nc.vector.memset(rs_z, 0.0)
            V12 = [sb.tile([128, H, HD + 1], bf16, name=f"V12_{s8}") for s8 in range(SC)]
            for s8 in range(SC):
                nc.vector.memset(V12[s8][:, :, HD:HD + 1], 1.0)

            # HAM warm-up: a short dummy-matmul burst from t=0 (while the
            # input DMAs run) flips the PE clock-gate to 8/8 just before the
            # transposes arrive; the dense transpose+QK stream keeps it warm.
            for _ in range(6):
                wu_ps = p2("wu_ps")
                for n0, n1 in HALVES:
                    nc.tensor.matmul(wu_ps[:, n0:n1], ones2d,
                                     rs_z[:, n0:n1], start=True, stop=True)

            # ---- input DMAs over three queues ----
            x_sb = sb.tile([128, SC, DIM], bf16, name="x_sb")
            nc.sync.dma_start(
                out=x_sb[:, 0:4, :],
                in_=x_ext[0:512, :].rearrange("(a p) n -> p a n", p=128))
            nc.scalar.dma_start(
                out=x_sb[:, 4:8, :],
                in_=x_ext[512:1024, :].rearrange("(a p) n -> p a n", p=128))

            bq_sb = sb.tile([128, KC], f32)
            nc.gpsimd.dma_start(out=bq_sb, in_=bq_ext[:].rearrange("(c p) -> p c", p=128))
            bk_sb = sb.tile([128, KC], f32)
            nc.gpsimd.dma_start(out=bk_sb, in_=bk_ext[:].rearrange("(c p) -> p c", p=128))
            bv_row = sb.tile([1, DIM], bf16)
            nc.gpsimd.dma_start(out=bv_row, in_=bv_ext[:].rearrange("(a d) -> a d", a=1))
            bp_row = sb.tile([1, DIM], bf16)
            nc.gpsimd.dma_start(out=bp_row, in_=bp_ext[:].rearrange("(a d) -> a d", a=1))

            def w_load(W_ext, eng, name):
                w = sb.tile([128, KC, DIM], bf16, name=name)
                eng.dma_start(
                    out=w, in_=W_ext[:].rearrange("(c p) n -> p c n", p=128))
                return [w[:, c, :] for c in range(KC)]

            Wq_sb = w_load(Wq_ext, nc.scalar, "Wq")
            Wk_sb = w_load(Wk_ext, nc.sync, "Wk")
            Wv_sb = w_load(Wv_ext, nc.scalar, "Wv")
            Wp_sb = w_load(Wp_ext, nc.gpsimd, "Wp")

            # x -> xT via PE transpose; PSUM->SBUF casts split DVE/ACT
            xT = [sb.tile([128, S], bf16, name=f"xT{c}") for c in range(KC)]
            for c in range(KC):
                xt_ps = p2("xt_ps", (128, S), bf16)
                for s8 in range(SC):
                    nc.tensor.transpose(
                        xt_ps[:, s8 * 128:(s8 + 1) * 128],
                        x_sb[:, s8, c * 128:(c + 1) * 128], ident)
                nc.vector.tensor_copy(xT[c][:, 0:512], xt_ps[:, 0:512])
                nc.scalar.copy(xT[c][:, 512:1024], xt_ps[:, 512:1024])

            # persistent activations
            QT = [sb.tile([128, S], bf16, name=f"QT{c}") for c in range(KC)]
            KT = [sb.tile([128, S], bf16, name=f"KT{c}") for c in range(KC)]

            # QT / KT chunk m: out[d_out, seq] = W^T @ xT, bias per partition
            def qk_chunk(W_sb, bias_sb, dst, m):
                q_ps = p2("q_ps")
                for k in range(KC):
                    for n0, n1 in HALVES:
                        nc.tensor.matmul(
                            q_ps[:, n0:n1],
                            W_sb[k][:, m * 128:(m + 1) * 128],
                            xT[k][:, n0:n1],
                            start=(k == 0), stop=(k == KC - 1))
                nc.vector.tensor_scalar_add(dst[m], q_ps, bias_sb[:, m:m + 1])

            # half-width QK piece: 6 matmuls, sized to fit the ~1.3us of PE
            # slack one exp-paced key-chunk slot offers
            def qk_half(W_sb, bias_sb, dst, m, half):
                n0, n1 = HALVES[half]
                q_ps = p2("qk_h", (128, 512))
                for k in range(KC):
                    nc.tensor.matmul(
                        q_ps, W_sb[k][:, m * 128:(m + 1) * 128],
                        xT[k][:, n0:n1],
                        start=(k == 0), stop=(k == KC - 1))
                nc.vector.tensor_scalar_add(dst[m][:, n0:n1], q_ps,
                                            bias_sb[:, m:m + 1])

            qk_chunk(Wq_sb, bq_sb, QT, 0)
            qk_chunk(Wk_sb, bk_sb, KT, 0)

            # broadcast bv/bp across partitions (K=128 matmul on the
            # zero-padded carrier; row 0 = bias row); after qk0 so nothing
            # in the PE FIFO waits on the slow gpsimd bias DMAs early on.
            bv_bc = sb.tile([128, DIM], f32)
            bp_bc = sb.tile([128, DIM], f32)
            for row, bc in ((bv_row, bv_bc), (bp_row, bp_bc)):
                nc.vector.tensor_copy(rs_z[0:1, 0:DIM], row[0:1, :])
                bc_ps = p2("bias_ps")
                for n0, n1 in VHALVES:
                    nc.tensor.matmul(bc_ps[:, n0:n1], ones2d,
                                     rs_z[:, n0:n1], start=True, stop=True)
                # DVE, not ACT: the strict-FIFO ACT queue must stay clear for
                # the first softmax exps
                nc.vector.tensor_copy(bc, bc_ps[:, 0:DIM])

            # V natural layout: out[seq, d] = x Wv (+bv); trickled into
            # pair 0's kc loop to fill the exp-paced PE slack.
            def emit_v_chunk(s8):
                v_ps = p2("v_ps")
                for k in range(KC):
                    for n0, n1 in VHALVES:
                        nc.tensor.matmul(
                            v_ps[:, n0:n1],
                            xT[k][:, s8 * 128:(s8 + 1) * 128],
                            Wv_sb[k][:, n0:n1],
                            start=(k == 0), stop=(k == KC - 1))
                nc.vector.tensor_add(
                    V12[s8][:, :, 0:HD],
                    v_ps[:, 0:DIM].rearrange("p (h d) -> p h d", h=H),
                    bv_bc[:].rearrange("p (h d) -> p h d", h=H))

            # ---- attention + output projection ----
            with tc.tile_pool(name="pb", bufs=1) as pb:
                OT = [pb.tile([128, S], bf16, name=f"OT{c}") for c in range(KC)]

                def norm_head(ov, c, half):
                    # rowsum (into row 0 of the zero-padded carrier) ->
                    # broadcast via full-util K=128 matmul -> 1/x -> normalize
                    nc.vector.tensor_copy(rs_z[0:1, :], ov[HD:HD + 1, :])
                    bc_ps = ps.tile([128, S], f32, tag="p2", bufs=2, name="bc_ps")
                    for n0, n1 in HALVES:
                        nc.tensor.matmul(bc_ps[:, n0:n1], ones2d,
                                         rs_z[:, n0:n1], start=True, stop=True)
                    rbc = pb.tile([HD, S], f32, tag="rbc", bufs=2, name="rbc")
                    nc.vector.reciprocal_approx_fast(rbc, bc_ps[0:HD, :])
                    base = half * HD
                    nc.vector.tensor_mul(OT[c][base:base + HD, :], ov[0:HD, :], rbc)

                def s_exp_part(c, pt, kcs, with_v=False, qk=None):
                    # S^T for head pair (2c, 2c+1): row-packed matmuls on array
                    # halves run concurrently; exp'd scores buffer in SBUF.
                    for kc in kcs:
                        st_e = ps.tile([128, S], f32, tag="st2", bufs=2, name="st_e")
                        st_o = ps.tile([128, S], f32, tag="st2", bufs=2, name="st_o")
                        for n0, n1 in HALVES:
                            nc.tensor.matmul(
                                st_e[:, n0:n1],
                                KT[c][0:HD, kc * 128:(kc + 1) * 128],
                                QT[c][0:HD, n0:n1],
                                start=True, stop=True)
                            nc.tensor.matmul(
                                st_o[:, n0:n1],
                                KT[c][HD:128, kc * 128:(kc + 1) * 128],
                                QT[c][HD:128, n0:n1],
                                start=True, stop=True)
                        p_e = pb.tile([128, S], bf16, tag=f"pt{kc}e", bufs=2, name=f"pt{kc}e")
                        nc.scalar.activation(p_e, st_e, EXP, scale=SCALE)
                        p_o = pb.tile([128, S], bf16, tag=f"pt{kc}o", bufs=2, name=f"pt{kc}o")
                        nc.scalar.activation(p_o, st_o, EXP, scale=SCALE)
                        pt[0].append(p_e)
                        pt[1].append(p_o)
                        if with_v:
                            emit_v_chunk(kc)
                        if qk is not None and kc in qk[1]:
                            # QK piece for the *next* pair, emitted inside
                            # this pair's exp-paced stretch: the next pair's
                            # scores then exist before ACT runs dry.
                            m, kind = qk[0], qk[1][kc]
                            if kind[0] == 'q':
                                qk_half(Wq_sb, bq_sb, QT, m, int(kind[1]))
                            else:
                                qk_half(Wk_sb, bk_sb, KT, m, int(kind[1]))
                    return pt

                def pv_piece(c, half, ptl, ov, kcs):
                    for kc in kcs:
                        for n0, n1 in HALVES:
                            nc.tensor.matmul(
                                ov[:, n0:n1],
                                V12[kc][:, 2 * c + half, :],
                                ptl[kc][:, n0:n1],
                                start=(kc == 0), stop=(kc == SC - 1))

                # Half-skewed pipeline (the ordering this hardware likes),
                # with a 2/6 split of each score stream across the iteration
                # boundary: key-chunks 0-1 of pair c+2 are emitted right
                # after pv(c,1), so ACT stays fed through the boundary while
                # the PE runs the next iteration's PV front.  QK chunks ride
                # inside the streams one pair ahead of use.
                pts_cur = ([], [])
                s_exp_part(0, pts_cur, range(SC), with_v=True,
                           qk=(1, {1: 'q0', 2: 'q1', 4: 'k0', 5: 'k1'}))
                pts_nxt = ([], [])
                s_exp_part(1, pts_nxt, range(0, 2))
                for c in range(KC):
                    nx = c + 1 < KC
                    qm = c + 2 if c + 2 < KC else None

                    def sp(kc, kind=None):
                        if nx:
                            s_exp_part(c + 1, pts_nxt, [kc],
                                       qk=(qm, {kc: kind})
                                       if (qm is not None and kind) else None)

                    # PV halves split into 8-matmul pieces laced between the
                    # exp-paced score chunks, so no PE block overruns the
                    # ~1.3us of slack one score slot offers.  The p2 pool
                    # acquisition order (ov0, bc0, q0, q1, ov1, bc1, k0, k1)
                    # keeps the 2-buffer rotation deadlock-free.
                    ov0 = ps.tile([HD + 1, S], f32, tag="p2", bufs=2, name="ov")
                    pv_piece(c, 0, pts_cur[0], ov0, range(0, 4))
                    sp(2)
                    pv_piece(c, 0, pts_cur[0], ov0, range(4, SC))
                    norm_head(ov0, c, 0)
                    sp(3, 'q0')
                    sp(4, 'q1')
                    ov1 = ps.tile([HD + 1, S], f32, tag="p2", bufs=2, name="ov")
                    pv_piece(c, 1, pts_cur[1], ov1, range(0, 4))
                    sp(5)
                    pv_piece(c, 1, pts_cur[1], ov1, range(4, SC))
                    norm_head(ov1, c, 1)
                    sp(6, 'k0')
                    sp(7, 'k1')
                    pts_fut = ([], [])
                    if c + 2 < KC:
                        s_exp_part(c + 2, pts_fut, range(0, 2))
                    pts_cur, pts_nxt = pts_nxt, pts_fut

                # out = O_cat @ Wp + bp; writeback halves on both HWDGE queues
                for s8 in range(SC):
                    f_ps = p2("f_ps")
                    for k in range(KC):
                        for n0, n1 in VHALVES:
                            nc.tensor.matmul(
                                f_ps[:, n0:n1],
                                OT[k][:, s8 * 128:(s8 + 1) * 128],
                                Wp_sb[k][:, n0:n1],
                                start=(k == 0), stop=(k == KC - 1))
                    # bf16 writeback (host upcasts): halves the output-DMA
                    # queue time that otherwise drains past the last matmul
                    fin = pb.tile([128, DIM], bf16, tag="fin", bufs=2, name="fin")
                    nc.vector.tensor_add(fin, f_ps[:, 0:DIM], bp_bc)
                    e0, e1 = (nc.sync, nc.scalar) if s8 % 2 == 0 else (nc.scalar, nc.sync)
                    e0.dma_start(out=out_ext[s8 * 128:(s8 + 1) * 128, 0:512],
                                 in_=fin[:, 0:512])
                    e1.dma_start(out=out_ext[s8 * 128:(s8 + 1) * 128, 512:DIM],
                                 in_=fin[:, 512:DIM])

    nc.compile()
    return nc


def get_nc():
    if "nc" not in _CACHE:
        _CACHE["nc"] = _build()
    return _CACHE["nc"]


def kernel(x, Wq, bq, Wk, bk, Wv, bv, Wp, bp):
    import ml_dtypes
    from concourse.bass_utils import run_bass_kernel_spmd

    nc = get_nc()
    bfl = ml_dtypes.bfloat16
    x = np.ascontiguousarray(np.asarray(x, np.float32).astype(bfl))
    shared = {
        "Wq": np.ascontiguousarray(np.asarray(Wq, np.float32).astype(bfl)),
        "bq": np.ascontiguousarray(np.asarray(bq, np.float32)),
        "Wk": np.ascontiguousarray(np.asarray(Wk, np.float32).astype(bfl)),
        "bk": np.ascontiguousarray(np.asarray(bk, np.float32)),
        "Wv": np.ascontiguousarray(np.asarray(Wv, np.float32).astype(bfl)),
        "bv": np.ascontiguousarray(np.asarray(bv, np.float32).astype(bfl)),
        "Wp": np.ascontiguousarray(np.asarray(Wp, np.float32).astype(bfl)),
        "bp": np.ascontiguousarray(np.asarray(bp, np.float32).astype(bfl)),
    }
    in_maps = [{"x": x[b], **shared} for b in range(N_CORES)]
    res = run_bass_kernel_spmd(nc, in_maps, core_ids=list(range(N_CORES)))
    return np.stack([res.results[i]["out"] for i in range(N_CORES)],
                    axis=0).astype(np.float32)
